# revision 5
# baseline (speedup 1.0000x reference)
"""Trainium2 Bass kernel for nn_Attention_63934883168998.

Math (per token t): q,k,v = x W{q,k,v}^T reshaped (16 heads, 64); scores over
HEADS: S = q k^T / 8 (16x16), A = softmax(S), out = A v -> (1024); y = out Wo^T.

Sharding: pure data parallel over the 16384 tokens -> 2048 tokens/core.
On-chip in fp16 (PE fp16 matmul = full rate; fp16 keeps GEMM rel-err ~5e-4).
Projections on PE: stationary = x^T chunk (via DMA transpose), moving = host-
pretransposed W^T (h,o).  The per-token 16x16 head attention runs on the
Vector engine via broadcast-AP multiplies + segmented reduces; softmax exp on
ScalarE.
"""

import numpy as np

N_CORES = 8
HID = 1024
NH, HD = 16, 64
TILE = 128
TPC = 16384 // N_CORES      # tokens per core
NT = TPC // TILE            # token tiles per core
NC_CHUNK = HID // 128       # 8 hidden chunks

_cache = {}


def _build():
    if "nc" in _cache:
        return
    import concourse.bacc as bacc
    import concourse.mybir as mybir
    from concourse import tile

    f16 = mybir.dt.float16
    f32 = mybir.dt.float32
    AX = mybir.AxisListType
    OP = mybir.AluOpType
    AF = mybir.ActivationFunctionType

    nc = bacc.Bacc("TRN2", target_bir_lowering=False, debug=False)
    xs = nc.dram_tensor("xs", (TPC, HID), f16, kind="ExternalInput").ap()
    wts = {
        n: nc.dram_tensor(n, (HID, HID), f16, kind="ExternalInput").ap()
        for n in ("wqt", "wkt", "wvt", "wot")
    }
    y = nc.dram_tensor("y", (TPC, HID), f32, kind="ExternalOutput").ap()

    with tile.TileContext(nc) as tc:
        with (
            tc.tile_pool(name="wpool", bufs=1) as wpool,
            tc.tile_pool(name="work", bufs=2) as work,
            tc.tile_pool(name="prod", bufs=1) as prodp,
            tc.tile_pool(name="psum", bufs=1, space="PSUM") as pp,
        ):
            # Resident weights, laid out (128, chunk, out) so chunk c is
            # W^T[c*128:(c+1)*128, :] with hidden-in on partitions.
            w_sb = {}
            for n in ("wqt", "wkt", "wvt", "wot"):
                wt = wpool.tile([128, NC_CHUNK, HID], f16, tag=n)
                nc.sync.dma_start(wt[:], wts[n].rearrange("(c p) o -> p c o", p=128))
                w_sb[n] = wt

            for it in range(NT):
                t0 = it * TILE
                # x^T chunks: (h_chunk 128, tokens 128) each, via DMA transpose
                xT = work.tile([128, NC_CHUNK, TILE], f16, tag="xT")
                for c in range(NC_CHUNK):
                    nc.sync.dma_start(
                        xT[:, c, :],
                        xs[t0 : t0 + TILE, c * 128 : (c + 1) * 128],
                        transpose=True,
                    )

                # q,k,v projections: psum[t, o_half] += xT_c^T @ W^T[c, half]
                ps = {
                    n: [pp.tile([128, 512], f32, name=f"ps{n}{h}", tag=f"ps{n}{h}") for h in range(2)]
                    for n in ("q", "k", "v")
                }
                for c in range(NC_CHUNK):
                    for n, wn in (("q", "wqt"), ("k", "wkt"), ("v", "wvt")):
                        for h in range(2):
                            nc.tensor.matmul(
                                ps[n][h][:],
                                xT[:, c, :],
                                w_sb[wn][:, c, h * 512 : (h + 1) * 512],
                                start=(c == 0),
                                stop=(c == NC_CHUNK - 1),
                            )

                q_sb = work.tile([128, HID], f16, tag="q")
                k_sb = work.tile([128, HID], f16, tag="k")
                # v stored d-major: (128, d 64, g 16) for the AV stage
                v_pm = work.tile([128, HD, NH], f16, tag="v")
                for h in range(2):
                    nc.scalar.copy(q_sb[:, h * 512 : (h + 1) * 512], ps["q"][h][:])
                    nc.scalar.copy(k_sb[:, h * 512 : (h + 1) * 512], ps["k"][h][:])
                    # psum v half h holds heads g=8h..8h+8 (g-major (g,d));
                    # write transposed into (d, g) layout
                    dst = v_pm[:, :, h * 8 : (h + 1) * 8]  # (128, 64, 8) strides (16,1)
                    src = ps["v"][h][:].rearrange("p (g d) -> p g d", g=8)
                    nc.scalar.copy(dst.rearrange("p d g -> p g d"), src)

                # scores: prod[t,(h,g,d)] = q[t,(h,d)] * k[t,(g,d)]; reduce d
                prod = prodp.tile([128, NH, NH, HD], f16, tag="prod")
                q_ap = (
                    q_sb[:]
                    .rearrange("p (h d) -> p h d", h=NH)
                    .unsqueeze(2)
                    .broadcast_to((128, NH, NH, HD))
                )
                k_ap = (
                    k_sb[:]
                    .rearrange("p (g d) -> p g d", g=NH)
                    .unsqueeze(1)
                    .broadcast_to((128, NH, NH, HD))
                )
                nc.vector.tensor_tensor(prod[:], q_ap, k_ap, op=OP.mult)
                # fp16 reduce output => all-2B operands => DVE 2x mode
                scores = work.tile([128, NH, NH], f16, tag="scores")
                with nc.allow_low_precision(reason="fp16 scores, rounded anyway"):
                    nc.vector.tensor_reduce(scores[:], prod[:], axis=AX.X, op=OP.add)

                # softmax over g (no max-subtract: logits ~N(0,1), exp safe)
                ex = work.tile([128, NH, NH], f16, tag="ex")
                nc.scalar.activation(ex[:], scores[:], AF.Exp, scale=0.125)
                ssum = work.tile([128, NH], f32, tag="ssum")
                nc.vector.tensor_reduce(ssum[:], ex[:], axis=AX.X, op=OP.add)
                rs = work.tile([128, NH], f32, tag="rs")
                nc.vector.reciprocal(rs[:], ssum[:])
                attw = work.tile([128, NH, NH], f16, tag="attw")
                nc.vector.tensor_tensor(
                    attw[:], ex[:], rs[:].unsqueeze(2).broadcast_to((128, NH, NH)),
                    op=OP.mult,
                )

                # AV: prod2[t,(h,d,g)] = A[t,(h,g)] * v[t,(d,g)]; reduce g
                prod2 = prodp.tile([128, NH, HD, NH], f16, tag="prod")
                a_ap = attw[:].unsqueeze(2).broadcast_to((128, NH, HD, NH))
                v_ap = v_pm[:].unsqueeze(1).broadcast_to((128, NH, HD, NH))
                nc.vector.tensor_tensor(prod2[:], a_ap, v_ap, op=OP.mult)
                attn16 = work.tile([128, HID], f16, tag="attn16")
                with nc.allow_low_precision(reason="fp16 attn, cast anyway"):
                    nc.vector.tensor_reduce(
                        attn16[:].rearrange("p (h d) -> p h d", h=NH),
                        prod2[:],
                        axis=AX.X,
                        op=OP.add,
                    )

                # output projection: oT chunks via SBUF->SBUF DMA transpose
                oT = work.tile([128, NC_CHUNK, TILE], f16, tag="oT")
                for c in range(NC_CHUNK):
                    nc.sync.dma_start(
                        oT[:, c, :], attn16[:, c * 128 : (c + 1) * 128], transpose=True
                    )
                py = [pp.tile([128, 512], f32, name=f"py{h}", tag=f"py{h}") for h in range(2)]
                for c in range(NC_CHUNK):
                    for h in range(2):
                        nc.tensor.matmul(
                            py[h][:],
                            oT[:, c, :],
                            w_sb["wot"][:, c, h * 512 : (h + 1) * 512],
                            start=(c == 0),
                            stop=(c == NC_CHUNK - 1),
                        )
                y_sb = work.tile([128, HID], f32, tag="ysb")
                for h in range(2):
                    nc.scalar.copy(y_sb[:, h * 512 : (h + 1) * 512], py[h][:])
                nc.sync.dma_start(y[t0 : t0 + TILE, :], y_sb[:])

    nc.compile()
    _cache["nc"] = nc


def _prep_inputs(x, wq, wk, wv, wo):
    x2 = np.asarray(x, dtype=np.float32).reshape(-1, HID)
    w16 = {
        n: np.ascontiguousarray(np.asarray(w, dtype=np.float32).T).astype(np.float16)
        for n, w in (("wqt", wq), ("wkt", wk), ("wvt", wv), ("wot", wo))
    }
    in_maps = []
    for i in range(N_CORES):
        sh = x2[i * TPC : (i + 1) * TPC].astype(np.float16)
        m = {"xs": np.ascontiguousarray(sh)}
        m.update(w16)
        in_maps.append(m)
    return in_maps


def kernel(x, wq, wk, wv, wo, _trace=False):
    from concourse import bass_utils

    _build()
    in_maps = _prep_inputs(x, wq, wk, wv, wo)
    res = bass_utils.run_bass_kernel_spmd(
        _cache["nc"], in_maps, core_ids=list(range(N_CORES)), trace=_trace
    )
    kernel.last_result = res
    B, S = 4, 4096
    out = np.concatenate([r["y"] for r in res.results], axis=0)
    return out.reshape(B, S, HID).astype(np.float32)


# revision 13
# speedup vs baseline: 1.5100x; 1.5100x over previous
"""Trainium2 Bass kernel for nn_Attention_63934883168998.

Math (per token t): q,k,v = x W{q,k,v}^T reshaped (16 heads, 64); scores over
HEADS: S = q k^T / 8 (16x16), A = softmax(S), out = A v -> (1024); y = out Wo^T.

Sharding: pure data parallel over the 16384 tokens -> 2048 tokens/core.
All on-chip data fp16 (PE fp16 matmul = full rate, ~5e-4 GEMM rel-err).

Per 128-token tile:
  - x^T arrives pre-transposed from the host; projections on PE with
    stationary = x^T chunk, moving = host-pretransposed W^T -> q,k,v in
    natural (token-partition) layout.
  - scores S[t,(g,h)] = sum_d q*k on the Vector engine: one broadcast-AP
    tensor_tensor (2x mode) + a log2(64)-pass pairwise-add tree (2x mode);
    softmax on ScalarE/Vector.  g-major layout so the A-scatter below has
    contiguous 32B runs.
  - AV combine on the TENSOR engine: stationary = 128x128 block-diagonal A
    for 8 tokens (K=(tau,g)), moving = v gathered to (tau,g)-partition
    layout; off-diagonal zeros kill cross-token terms.  Operands built by
    partition-scatter DMAs (plain, 32B/128B runs).
  - attention out scattered back to token-partition layout, DMA-transposed
    to feature-partition chunks, final projection on PE.
"""

import numpy as np

N_CORES = 8
HID = 1024
NH, HD = 16, 64
TILE = 128
TPC = 16384 // N_CORES      # tokens per core
NT = TPC // TILE            # token tiles per core
NCH = HID // 128            # 8 hidden chunks
NG = TILE // 8              # 16 groups of 8 tokens

_cache = {}


def _build():
    if "nc" in _cache:
        return
    import concourse.bacc as bacc
    import concourse.mybir as mybir
    from concourse import tile

    f16 = mybir.dt.float16
    f32 = mybir.dt.float32
    AX = mybir.AxisListType
    OP = mybir.AluOpType
    AF = mybir.ActivationFunctionType

    nc = bacc.Bacc("TRN2", target_bir_lowering=False, debug=False)
    xt = nc.dram_tensor("xt", (HID, TPC), f16, kind="ExternalInput").ap()
    wts = {
        n: nc.dram_tensor(n, (HID, HID), f16, kind="ExternalInput").ap()
        for n in ("wqt", "wkt", "wvt", "wot")
    }
    # block-diag 0/1 mask: mask[(b,g),(b',h)] = (b == b')
    mask_d = nc.dram_tensor("mask", (128, 128), f16, kind="ExternalInput").ap()
    y = nc.dram_tensor("y", (TPC, HID), f32, kind="ExternalOutput").ap()

    with tile.TileContext(nc) as tc:
        with (
            tc.tile_pool(name="wpool", bufs=1) as wpool,
            tc.tile_pool(name="work", bufs=2) as work,
            tc.tile_pool(name="prod", bufs=1) as prodp,
            tc.tile_pool(name="psum", bufs=1, space="PSUM") as pp,
        ):
            # Resident weights: chunk c = W^T[c*128:(c+1)*128, :]
            w_sb = {}
            for n in ("wqt", "wkt", "wvt", "wot"):
                wt = wpool.tile([128, NCH, HID], f16, tag=n)
                nc.sync.dma_start(wt[:], wts[n].rearrange("(c p) o -> p c o", p=128))
                w_sb[n] = wt

            mask_sb = wpool.tile([128, 128], f16, tag="mask")
            nc.sync.dma_start(mask_sb[:], mask_d[:])

            xt_r = xt.rearrange("(c p) t -> p c t", p=128)

            for it in range(NT):
                t0 = it * TILE
                xT = work.tile([128, NCH, TILE], f16, tag="xT")
                nc.sync.dma_start(xT[:], xt_r[:, :, t0 : t0 + TILE])

                # ---- projections q,k,v ----
                ps = {
                    n: [pp.tile([128, 512], f32, name=f"ps{n}{h}", tag=f"ps{n}{h}")
                        for h in range(2)]
                    for n in ("q", "k", "v")
                }
                for c in range(NCH):
                    for n, wn in (("q", "wqt"), ("k", "wkt"), ("v", "wvt")):
                        for h in range(2):
                            nc.tensor.matmul(
                                ps[n][h][:],
                                xT[:, c, :],
                                w_sb[wn][:, c, h * 512 : (h + 1) * 512],
                                start=(c == 0),
                                stop=(c == NCH - 1),
                            )
                q_sb = work.tile([128, HID], f16, tag="q")
                k_sb = work.tile([128, HID], f16, tag="k")
                v_sb = work.tile([128, HID], f16, tag="v")
                for h in range(2):
                    nc.scalar.copy(q_sb[:, h * 512 : (h + 1) * 512], ps["q"][h][:])
                    nc.scalar.copy(k_sb[:, h * 512 : (h + 1) * 512], ps["k"][h][:])
                    nc.scalar.copy(v_sb[:, h * 512 : (h + 1) * 512], ps["v"][h][:])

                # ---- scores, g-major: prod[t,(g,h,d)] = k[t,(g,d)] * q[t,(h,d)]
                prod = prodp.tile([128, NH, NH, HD], f16, tag="prod")
                q_ap = (
                    q_sb[:]
                    .rearrange("p (h d) -> p h d", h=NH)
                    .unsqueeze(1)
                    .broadcast_to((128, NH, NH, HD))
                )
                k_ap = (
                    k_sb[:]
                    .rearrange("p (g d) -> p g d", g=NH)
                    .unsqueeze(2)
                    .broadcast_to((128, NH, NH, HD))
                )
                nc.vector.tensor_tensor(prod[:], k_ap, q_ap, op=OP.mult)

                # pairwise-add tree over d (all ops 2B + step1 => DVE 2x)
                p3 = prod[:].rearrange("p g h d -> p (g h) d")
                scrA = prodp.tile([128, NH * NH, 32], f16, tag="scrA")
                scrB = prodp.tile([128, NH * NH, 16], f16, tag="scrB")
                with nc.allow_low_precision(reason="fp16 score partials"):
                    nc.vector.tensor_tensor(
                        scrA[:], p3[:, :, 0:32], p3[:, :, 32:64], op=OP.add
                    )
                    nc.vector.tensor_tensor(
                        scrB[:], scrA[:, :, 0:16], scrA[:, :, 16:32], op=OP.add
                    )
                    nc.vector.tensor_tensor(
                        scrA[:, :, 0:8], scrB[:, :, 0:8], scrB[:, :, 8:16], op=OP.add
                    )
                    nc.vector.tensor_tensor(
                        scrB[:, :, 0:4], scrA[:, :, 0:4], scrA[:, :, 4:8], op=OP.add
                    )
                    nc.vector.tensor_tensor(
                        scrA[:, :, 0:2], scrB[:, :, 0:2], scrB[:, :, 2:4], op=OP.add
                    )
                    scores = work.tile([128, NH * NH], f16, tag="scores")
                    nc.vector.tensor_tensor(
                        scores[:].unsqueeze(2),
                        scrA[:, :, 0:1],
                        scrA[:, :, 1:2],
                        op=OP.add,
                    )

                # ---- softmax over g (scores laid out (g,h)) ----
                ex = work.tile([128, NH * NH], f16, tag="ex")
                nc.scalar.activation(ex[:], scores[:], AF.Exp, scale=0.125)
                ssum = work.tile([128, NH], f32, tag="ssum")
                ex_hg = ex[:].rearrange("p (g h) -> p h g", g=NH)  # strided view
                nc.vector.tensor_reduce(ssum[:], ex_hg, axis=AX.X, op=OP.add)
                rs = work.tile([128, NH], f32, tag="rs")
                nc.vector.reciprocal(rs[:], ssum[:])
                attw = work.tile([128, NH, NH], f16, tag="attw")  # (g, h)
                nc.vector.tensor_tensor(
                    attw[:],
                    ex[:].rearrange("p (g h) -> p g h", g=NH),
                    rs[:].unsqueeze(1).broadcast_to((128, NH, NH)),
                    op=OP.mult,
                )

                # ---- AV on PE ----
                # K-partition index (b,g), b = token-within-contiguous-8-group.
                # Scatter A rows to (b,g)-partition compact layout, gather v
                # rows likewise; DVE broadcasts A over b' and masks to
                # block-diagonal; one 128x128 @ 128x64 matmul = 8 tokens.
                a_k = work.tile([128, NG, NH], f16, tag="a_k")
                vS = work.tile([128, NG, HD], f16, tag="vS")
                for grp in range(NG):
                    # A_k[(b,g), grp, h] = attw[grp*8+b, g, h]
                    eng = nc.scalar if grp % 2 == 0 else nc.sync
                    eng.dma_start(a_k[:, grp, :], attw[grp * 8 : (grp + 1) * 8, :, :])
                    # vS[(b,g), grp, d] = v[grp*8+b, (g,d)]
                    src_v = v_sb[grp * 8 : (grp + 1) * 8, :].rearrange(
                        "t (g d) -> t g d", g=NH
                    )
                    eng2 = nc.sync if grp % 2 == 0 else nc.scalar
                    eng2.dma_start(vS[:, grp, :], src_v)
                abd_m = prodp.tile([128, NG, 8, NH], f16, tag="abd_m")
                nc.vector.tensor_tensor(
                    abd_m[:],
                    a_k[:].unsqueeze(2).broadcast_to((128, NG, 8, NH)),
                    mask_sb[:]
                    .rearrange("p (b h) -> p b h", b=8)
                    .unsqueeze(1)
                    .broadcast_to((128, NG, 8, NH)),
                    op=OP.mult,
                )
                pa = [pp.tile([128, NG // 2, HD], f32, name=f"pa{i}", tag=f"pav{i}")
                      for i in range(2)]
                for grp in range(NG):
                    nc.tensor.matmul(
                        pa[grp // 8][:, grp % 8, :],
                        abd_m[:, grp, :, :].rearrange("p b h -> p (b h)"),
                        vS[:, grp, :],
                        start=True,
                        stop=True,
                    )
                # attn in ((b,h), grp, d) partition-interleaved layout
                attn_pm = work.tile([128, NG, HD], f16, tag="attn_pm")
                for i in range(2):
                    nc.scalar.copy(attn_pm[:, i * 8 : (i + 1) * 8, :], pa[i][:])

                # scatter back to token-partition natural layout
                attn16 = work.tile([128, HID], f16, tag="attn16")
                for grp in range(NG):
                    eng = nc.scalar if grp % 2 == 0 else nc.sync
                    eng.dma_start(
                        attn16[grp * 8 : (grp + 1) * 8, :].rearrange(
                            "t (h d) -> t h d", h=NH
                        ),
                        attn_pm[:, grp, :],
                    )

                # ---- output projection ----
                oT = work.tile([128, NCH, TILE], f16, tag="oT")
                for c in range(NCH):
                    eng = nc.sync if c % 2 == 0 else nc.scalar
                    eng.dma_start(
                        oT[:, c, :], attn16[:, c * 128 : (c + 1) * 128], transpose=True
                    )
                py = [pp.tile([128, 512], f32, name=f"py{h}", tag=f"pav{h}")
                      for h in range(2)]
                for c in range(NCH):
                    for h in range(2):
                        nc.tensor.matmul(
                            py[h][:],
                            oT[:, c, :],
                            w_sb["wot"][:, c, h * 512 : (h + 1) * 512],
                            start=(c == 0),
                            stop=(c == NCH - 1),
                        )
                y_sb = work.tile([128, HID], f32, tag="ysb")
                for h in range(2):
                    nc.scalar.copy(y_sb[:, h * 512 : (h + 1) * 512], py[h][:])
                nc.sync.dma_start(y[t0 : t0 + TILE, :], y_sb[:])

    nc.compile()
    _cache["nc"] = nc


def _prep_inputs(x, wq, wk, wv, wo):
    x2 = np.asarray(x, dtype=np.float32).reshape(-1, HID)
    w16 = {
        n: np.ascontiguousarray(np.asarray(w, dtype=np.float32).T).astype(np.float16)
        for n, w in (("wqt", wq), ("wkt", wk), ("wvt", wv), ("wot", wo))
    }
    mask = np.zeros((8, 16, 8, 16), dtype=np.float16)
    for b in range(8):
        mask[b, :, b, :] = 1.0
    mask = mask.reshape(128, 128)
    in_maps = []
    for i in range(N_CORES):
        sh = x2[i * TPC : (i + 1) * TPC].astype(np.float16)
        m = {"xt": np.ascontiguousarray(sh.T), "mask": mask}
        m.update(w16)
        in_maps.append(m)
    return in_maps


def kernel(x, wq, wk, wv, wo, _trace=False):
    from concourse import bass_utils

    _build()
    in_maps = _prep_inputs(x, wq, wk, wv, wo)
    res = bass_utils.run_bass_kernel_spmd(
        _cache["nc"], in_maps, core_ids=list(range(N_CORES)), trace=_trace
    )
    kernel.last_result = res
    B, S = 4, 4096
    out = np.concatenate([r["y"] for r in res.results], axis=0)
    return out.reshape(B, S, HID).astype(np.float32)


# revision 16
# speedup vs baseline: 1.5186x; 1.0057x over previous
"""Trainium2 Bass kernel for nn_Attention_63934883168998.

Math (per token t): q,k,v = x W{q,k,v}^T reshaped (16 heads, 64); scores over
HEADS: S = q k^T / 8 (16x16), A = softmax(S), out = A v -> (1024); y = out Wo^T.

Sharding: pure data parallel over the 16384 tokens -> 2048 tokens/core.
All on-chip data fp16 (PE fp16 matmul = full rate, ~5e-4 GEMM rel-err).

Per 128-token tile:
  - x^T arrives pre-transposed from the host; projections on PE with
    stationary = x^T chunk, moving = host-pretransposed W^T -> q,k,v in
    natural (token-partition) layout.
  - scores S[t,(g,h)] = sum_d q*k on the Vector engine: one broadcast-AP
    tensor_tensor (2x mode) + a log2(64)-pass pairwise-add tree (2x mode);
    softmax on ScalarE/Vector.  g-major layout so the A-scatter below has
    contiguous 32B runs.
  - AV combine on the TENSOR engine: stationary = 128x128 block-diagonal A
    for 8 tokens (K=(tau,g)), moving = v gathered to (tau,g)-partition
    layout; off-diagonal zeros kill cross-token terms.  Operands built by
    partition-scatter DMAs (plain, 32B/128B runs).
  - attention out scattered back to token-partition layout, DMA-transposed
    to feature-partition chunks, final projection on PE.
"""

import numpy as np

N_CORES = 8
HID = 1024
NH, HD = 16, 64
TILE = 128
TPC = 16384 // N_CORES      # tokens per core
NT = TPC // TILE            # token tiles per core
NCH = HID // 128            # 8 hidden chunks
NG = TILE // 8              # 16 groups of 8 tokens

_cache = {}


def _build():
    if "nc" in _cache:
        return
    import concourse.bacc as bacc
    import concourse.mybir as mybir
    from concourse import tile

    f16 = mybir.dt.float16
    f32 = mybir.dt.float32
    AX = mybir.AxisListType
    OP = mybir.AluOpType
    AF = mybir.ActivationFunctionType

    nc = bacc.Bacc("TRN2", target_bir_lowering=False, debug=False)
    xt = nc.dram_tensor("xt", (HID, TPC), f16, kind="ExternalInput").ap()
    wts = {
        n: nc.dram_tensor(n, (HID, HID), f16, kind="ExternalInput").ap()
        for n in ("wqt", "wkt", "wvt", "wot")
    }
    # block-diag 0/1 mask: mask[(b,g),(b',h)] = (b == b')
    mask_d = nc.dram_tensor("mask", (128, 128), f16, kind="ExternalInput").ap()
    y = nc.dram_tensor("y", (TPC, HID), f32, kind="ExternalOutput").ap()

    with tile.TileContext(nc) as tc:
        with (
            tc.tile_pool(name="wpool", bufs=1) as wpool,
            tc.tile_pool(name="work", bufs=2) as work,
            tc.tile_pool(name="prod", bufs=1) as prodp,
            tc.tile_pool(name="psum", bufs=1, space="PSUM") as pp,
        ):
            # Resident weights: chunk c = W^T[c*128:(c+1)*128, :]
            w_sb = {}
            for n in ("wqt", "wkt", "wvt", "wot"):
                wt = wpool.tile([128, NCH, HID], f16, tag=n)
                nc.sync.dma_start(wt[:], wts[n].rearrange("(c p) o -> p c o", p=128))
                w_sb[n] = wt

            mask_sb = wpool.tile([128, 128], f16, tag="mask")
            nc.sync.dma_start(mask_sb[:], mask_d[:])

            xt_r = xt.rearrange("(c p) t -> p c t", p=128)

            for it in range(NT):
                t0 = it * TILE
                xT = work.tile([128, NCH, TILE], f16, tag="xT")
                nc.sync.dma_start(xT[:], xt_r[:, :, t0 : t0 + TILE])

                # ---- projections q,k,v ----
                ps = {
                    n: [pp.tile([128, 512], f32, name=f"ps{n}{h}", tag=f"ps{n}{h}")
                        for h in range(2)]
                    for n in ("q", "k", "v")
                }
                for c in range(NCH):
                    for n, wn in (("q", "wqt"), ("k", "wkt"), ("v", "wvt")):
                        for h in range(2):
                            nc.tensor.matmul(
                                ps[n][h][:],
                                xT[:, c, :],
                                w_sb[wn][:, c, h * 512 : (h + 1) * 512],
                                start=(c == 0),
                                stop=(c == NCH - 1),
                            )
                q_sb = work.tile([128, HID], f16, tag="q")
                k_sb = work.tile([128, HID], f16, tag="k")
                # comb packs [A (16) | v (64)] per head-group g so one scatter
                # DMA per token-group moves both to (b,g)-partition layout
                comb = work.tile([128, NH, 16 + HD], f16, tag="comb")
                for h in range(2):
                    nc.scalar.copy(q_sb[:, h * 512 : (h + 1) * 512], ps["q"][h][:])
                    nc.scalar.copy(k_sb[:, h * 512 : (h + 1) * 512], ps["k"][h][:])
                    nc.scalar.copy(
                        comb[:, h * 8 : (h + 1) * 8, 16:],
                        ps["v"][h][:].rearrange("p (g d) -> p g d", g=8),
                    )
                v_sb = comb[:, :, 16:]

                # ---- scores, g-major: prod[t,(g,h,d)] = k[t,(g,d)] * q[t,(h,d)]
                prod = prodp.tile([128, NH, NH, HD], f16, tag="prod")
                q_ap = (
                    q_sb[:]
                    .rearrange("p (h d) -> p h d", h=NH)
                    .unsqueeze(1)
                    .broadcast_to((128, NH, NH, HD))
                )
                k_ap = (
                    k_sb[:]
                    .rearrange("p (g d) -> p g d", g=NH)
                    .unsqueeze(2)
                    .broadcast_to((128, NH, NH, HD))
                )
                nc.vector.tensor_tensor(prod[:], k_ap, q_ap, op=OP.mult)

                # pairwise-add tree over d (all ops 2B + step1 => DVE 2x)
                p3 = prod[:].rearrange("p g h d -> p (g h) d")
                scrA = prodp.tile([128, NH * NH, 32], f16, tag="scrA")
                scrB = prodp.tile([128, NH * NH, 16], f16, tag="scrB")
                with nc.allow_low_precision(reason="fp16 score partials"):
                    nc.vector.tensor_tensor(
                        scrA[:], p3[:, :, 0:32], p3[:, :, 32:64], op=OP.add
                    )
                    nc.vector.tensor_tensor(
                        scrB[:], scrA[:, :, 0:16], scrA[:, :, 16:32], op=OP.add
                    )
                    nc.vector.tensor_tensor(
                        scrA[:, :, 0:8], scrB[:, :, 0:8], scrB[:, :, 8:16], op=OP.add
                    )
                    nc.vector.tensor_tensor(
                        scrB[:, :, 0:4], scrA[:, :, 0:4], scrA[:, :, 4:8], op=OP.add
                    )
                    nc.vector.tensor_tensor(
                        scrA[:, :, 0:2], scrB[:, :, 0:2], scrB[:, :, 2:4], op=OP.add
                    )
                    scores = work.tile([128, NH * NH], f16, tag="scores")
                    nc.vector.tensor_tensor(
                        scores[:].unsqueeze(2),
                        scrA[:, :, 0:1],
                        scrA[:, :, 1:2],
                        op=OP.add,
                    )

                # ---- softmax over g (scores laid out (g,h)) ----
                ex = work.tile([128, NH * NH], f16, tag="ex")
                nc.scalar.activation(ex[:], scores[:], AF.Exp, scale=0.125)
                ssum = work.tile([128, NH], f32, tag="ssum")
                ex_hg = ex[:].rearrange("p (g h) -> p h g", g=NH)  # strided view
                nc.vector.tensor_reduce(ssum[:], ex_hg, axis=AX.X, op=OP.add)
                rs = work.tile([128, NH], f32, tag="rs")
                nc.vector.reciprocal(rs[:], ssum[:])
                attw = comb[:, :, 0:16]  # (g, h) slot of comb
                nc.vector.tensor_tensor(
                    attw,
                    ex[:].rearrange("p (g h) -> p g h", g=NH),
                    rs[:].unsqueeze(1).broadcast_to((128, NH, NH)),
                    op=OP.mult,
                )

                # ---- AV on PE ----
                # K-partition index (b,g), b = token-within-contiguous-8-group.
                # One scatter per 8-token group moves [A|v] to (b,g)-partition
                # layout (SWDGE queue); DVE broadcasts A over b' and masks to
                # block-diagonal; one 128x128 @ 128x64 matmul = 8 tokens.
                comb_k = work.tile([128, NG, 16 + HD], f16, tag="comb_k")
                for grp in range(NG):
                    nc.gpsimd.dma_start(
                        comb_k[:, grp, :], comb[grp * 8 : (grp + 1) * 8, :, :]
                    )
                abd_m = prodp.tile([128, NG, 8, NH], f16, tag="abd_m")
                nc.vector.tensor_tensor(
                    abd_m[:],
                    comb_k[:, :, 0:16].unsqueeze(2).broadcast_to((128, NG, 8, NH)),
                    mask_sb[:]
                    .rearrange("p (b h) -> p b h", b=8)
                    .unsqueeze(1)
                    .broadcast_to((128, NG, 8, NH)),
                    op=OP.mult,
                )
                pa = [pp.tile([128, NG // 2, HD], f32, name=f"pa{i}", tag=f"pav{i}")
                      for i in range(2)]
                for grp in range(NG):
                    nc.tensor.matmul(
                        pa[grp // 8][:, grp % 8, :],
                        abd_m[:, grp, :, :].rearrange("p b h -> p (b h)"),
                        comb_k[:, grp, 16:],
                        start=True,
                        stop=True,
                    )
                # attn in ((b,h), grp, d) partition-interleaved layout
                attn_pm = work.tile([128, NG, HD], f16, tag="attn_pm")
                for i in range(2):
                    nc.scalar.copy(attn_pm[:, i * 8 : (i + 1) * 8, :], pa[i][:])

                # scatter back to token-partition natural layout
                attn16 = work.tile([128, HID], f16, tag="attn16")
                for grp in range(NG):
                    nc.gpsimd.dma_start(
                        attn16[grp * 8 : (grp + 1) * 8, :].rearrange(
                            "t (h d) -> t h d", h=NH
                        ),
                        attn_pm[:, grp, :],
                    )

                # ---- output projection ----
                oT = work.tile([128, NCH, TILE], f16, tag="oT")
                for c in range(NCH):
                    eng = nc.sync if c % 2 == 0 else nc.scalar
                    eng.dma_start(
                        oT[:, c, :], attn16[:, c * 128 : (c + 1) * 128], transpose=True
                    )
                py = [pp.tile([128, 512], f32, name=f"py{h}", tag=f"pav{h}")
                      for h in range(2)]
                for c in range(NCH):
                    for h in range(2):
                        nc.tensor.matmul(
                            py[h][:],
                            oT[:, c, :],
                            w_sb["wot"][:, c, h * 512 : (h + 1) * 512],
                            start=(c == 0),
                            stop=(c == NCH - 1),
                        )
                y_sb = work.tile([128, HID], f32, tag="ysb")
                for h in range(2):
                    nc.scalar.copy(y_sb[:, h * 512 : (h + 1) * 512], py[h][:])
                nc.sync.dma_start(y[t0 : t0 + TILE, :], y_sb[:])

    nc.compile()
    _cache["nc"] = nc


def _prep_inputs(x, wq, wk, wv, wo):
    x2 = np.asarray(x, dtype=np.float32).reshape(-1, HID)
    w16 = {
        n: np.ascontiguousarray(np.asarray(w, dtype=np.float32).T).astype(np.float16)
        for n, w in (("wqt", wq), ("wkt", wk), ("wvt", wv), ("wot", wo))
    }
    mask = np.zeros((8, 16, 8, 16), dtype=np.float16)
    for b in range(8):
        mask[b, :, b, :] = 1.0
    mask = mask.reshape(128, 128)
    in_maps = []
    for i in range(N_CORES):
        sh = x2[i * TPC : (i + 1) * TPC].astype(np.float16)
        m = {"xt": np.ascontiguousarray(sh.T), "mask": mask}
        m.update(w16)
        in_maps.append(m)
    return in_maps


def kernel(x, wq, wk, wv, wo, _trace=False):
    from concourse import bass_utils

    _build()
    in_maps = _prep_inputs(x, wq, wk, wv, wo)
    res = bass_utils.run_bass_kernel_spmd(
        _cache["nc"], in_maps, core_ids=list(range(N_CORES)), trace=_trace
    )
    kernel.last_result = res
    B, S = 4, 4096
    out = np.concatenate([r["y"] for r in res.results], axis=0)
    return out.reshape(B, S, HID).astype(np.float32)
